# revision 24
# baseline (speedup 1.0000x reference)
"""Trainium2 Bass kernel for the LeNet C3 dense-conv layer.

Computes out = conv2d_valid(x, K, stride 1) + bias where K is the dense
[16, 6, 5, 5] kernel scattered from the sparse per-branch weights
(w3/w4/w6), x is [128, 6, 256, 256] f32, out is [128, 16, 252, 252] f32.

Strategy:
  - Pure data parallelism: 16 images per NeuronCore across 8 cores.
  - The conv is computed as shift-accumulated banded matmuls into PSUM.
    A block covers 6 output rows of FOUR images: the contraction dim
    stacks TWO copies of the 10 input rows (60 partitions each), the
    second copy pre-shifted one column, so each matmul covers two kernel
    columns kx at once: 3 matmuls per image-pair group (kx {0,1}, {2,3},
    {4}) instead of 5. The lhsT is a host-built banded matrix [120, 96]
    whose column m = c_out*6 + r holds K_dense[c_out, c_in, i-r, kx] at
    row i*6 + c_in (+60 for the kx+1 band). Each matmul's moving operand
    carries an image pair (N = 2*256 = 512 = one PSUM bank); the 4-image
    tile feeds two PSUM groups from one input DMA.
  - fp16 operands (~3e-4 rel err; accumulation is fp32 in PSUM). The PE
    here streams 1 column/cycle at 1.2 GHz regardless of dtype, so
    fewer matmul columns == faster; HWDGE rings cost ~9 ns/descriptor,
    so fewer/larger DMAs == faster.
  - Host pre-packs x into x4[q, (h,c), j*256+w] fp16 (2 KB descriptors,
    >=16 outer units per DMA striped over all 16 SDMA engines) and
    un-packs the device output o4[q, c, h, j*252+w] f32 (one 378 KB
    output DMA per block, 4 KB descriptors).
  - PSUM eviction on the vector engine fuses the per-partition bias add.
    The 4 leading pad columns of each input tile keep the 512-wide
    moving operand in-bounds; pads feed only discarded PSUM columns
    (interior images' "pads" are the previous image's tail, same deal).
"""

import numpy as np

# LeNet-5 C3 sparse channel connectivity (from the model definition).
CH3 = np.array([[0, 1, 2], [1, 2, 3], [2, 3, 4], [3, 4, 5], [0, 4, 5], [0, 1, 5]])
CH4 = np.array([[0, 1, 2, 3], [1, 2, 3, 4], [2, 3, 4, 5], [0, 3, 4, 5],
                [0, 1, 4, 5], [0, 1, 2, 5], [0, 1, 3, 4], [1, 2, 4, 5],
                [0, 2, 3, 5]])

B, C, H, W = 128, 6, 256, 256
CO, HO, WO = 16, 252, 252
NCORES = 8
BPC = B // NCORES           # images per core
NQ = BPC // 4               # 4-image groups per core
KH = KW = 5

R = 6                       # output rows per block
HI = R + 4                  # input rows per block
NBLK = HO // R              # 42 blocks per image quad
KK = C * HI                 # contraction rows per kx copy (60)
MM = CO * R                 # psum partitions (96)

_STATE = None  # cached Bass module so repeat kernel() calls skip re-tracing


def _dense_kernel(w3, w4, w6):
    k = np.zeros((CO, C, KH, KW), np.float32)
    k[np.arange(6)[:, None], CH3] = w3
    k[6 + np.arange(9)[:, None], CH4] = w4
    k[15] = w6[0]
    return k


def _band(kd, kx):
    """Banded lhsT [KK, MM] for kernel column kx: row i*6 + c_in,
    column c_out*R + r, value kd[c_out, c_in, i-r, kx]."""
    out = np.zeros((KK, MM), np.float32)
    for ci in range(C):
        for i in range(HI):
            for r in range(R):
                ky = i - r
                if 0 <= ky < KH:
                    out[i * C + ci, np.arange(CO) * R + r] = kd[:, ci, ky, kx]
    return out


def _build_module():
    import concourse.bacc as bacc
    import concourse.mybir as mybir
    from concourse.tile import TileContext

    f32 = mybir.dt.float32
    f16 = mybir.dt.float16

    # Bacc (not Bass): its compile() runs generate_event_semaphores(),
    # which splits multi-wait instructions to satisfy the TRN2 1-wait-
    # per-instruction constraint walrus enforces.
    nc = bacc.Bacc(None)
    # x4[q, (h, c), j*256 + w] = x[4q + j, c, h, w]  (host pre-pack)
    x_d = nc.dram_tensor("x", [NQ, H * C, 4 * W], f16, kind="ExternalInput")
    # wall: [120, 3*96] = [B(0); B(1)] | [B(2); B(3)] | [B(4); 0].
    # All three lhsTs share one tile with uniform K=120 so consecutive
    # LDWEIGHTS keep the same active-row config and pipeline with MMs.
    wall_d = nc.dram_tensor("wall", [2 * KK, 3 * MM], f16, kind="ExternalInput")
    b1_d = nc.dram_tensor("b1", [MM, 1], f32, kind="ExternalInput")
    # o4[q, c, h, j*252 + w] = out[4q + j, c, h, w]  (host un-packs)
    o_d = nc.dram_tensor("o", [NQ, CO, HO, 4 * WO], f32, kind="ExternalOutput")

    with TileContext(nc) as tc:
        with (
            tc.tile_pool(name="wpool", bufs=1) as wp,
            tc.tile_pool(name="inpool", bufs=8) as ip,
            tc.tile_pool(name="outpool", bufs=6) as op,
            tc.tile_pool(name="pspool", bufs=6, space="PSUM") as pp,
        ):
            wall_t = wp.tile([2 * KK, 3 * MM], f16)
            nc.sync.dma_start(wall_t[:], wall_d[:])
            b1_t = wp.tile([MM, 1], f32)
            nc.sync.dma_start(b1_t[:], b1_d[:])

            # Prime each constant tile on its consuming engine class so
            # steady-state instructions carry few semaphore waits.
            prime_ps = pp.tile([MM, 192], f32, tag="ps")
            nc.tensor.matmul(prime_ps[:], wall_t[:, 0:MM], wall_t[:, 0:192],
                             start=True, stop=True)
            prime_b = op.tile([MM, 1], f32, tag="out")
            nc.vector.tensor_scalar_add(prime_b[:], b1_t[:], 0.0)

            for q in range(NQ):
                for g in range(NBLK):
                    h0 = R * g

                    # Input tile: rows 0..59 hold x cols at t-4 (copy 1),
                    # rows 60..119 the same data at t-3 (copy 2, i.e.
                    # pre-shifted one column for the kx+1 band).
                    it = ip.tile([2 * KK, 4 + 4 * W], f16, tag="in")
                    src = x_d[q, h0 * C:(h0 + HI) * C, :]
                    nc.sync.dma_start(it[0:KK, 4:4 + 4 * W], src)
                    nc.sync.dma_start(it[KK:2 * KK, 3:3 + 4 * W], src)

                    ot = op.tile([MM, 4 * WO], f32, tag="out")
                    for grp in range(2):        # image pairs (0,1), (2,3)
                        base = 512 * grp
                        ps = pp.tile([MM, 2, 256], f32, tag="ps")
                        # q=0 -> kx {0,1}; q=2 -> kx {2,3}; q=4 -> kx 4.
                        nc.tensor.matmul(ps[:, :, :], wall_t[:, 0:MM],
                                         it[:, base:base + 512],
                                         start=True, stop=False)
                        nc.tensor.matmul(ps[:, :, :], wall_t[:, MM:2 * MM],
                                         it[:, base + 2:base + 514],
                                         start=False, stop=False)
                        nc.tensor.matmul(ps[:, :, :], wall_t[:, 2 * MM:3 * MM],
                                         it[:, base + 4:base + 516],
                                         start=False, stop=True)
                        nc.vector.tensor_scalar_add(
                            ot[:, 2 * WO * grp:2 * WO * (grp + 1)]
                            .rearrange("p (j w) -> p j w", j=2),
                            ps[:, :, 4:4 + WO],
                            b1_t[:, 0:1],
                        )
                    nc.scalar.dma_start(o_d[q, :, h0:h0 + R, :], ot[:])
    nc.compile()
    return nc


def _get_module():
    global _STATE
    if _STATE is None:
        _STATE = _build_module()
    return _STATE


def kernel(x, w3, b3, w4, b4, w6, b6):
    from concourse.bass_utils import run_bass_kernel_spmd

    x = np.asarray(x, np.float32)
    kd = _dense_kernel(np.asarray(w3, np.float32), np.asarray(w4, np.float32),
                       np.asarray(w6, np.float32))
    bias = np.concatenate([np.asarray(b3, np.float32),
                           np.asarray(b4, np.float32),
                           np.asarray(b6, np.float32)])

    zero = np.zeros((KK, MM), np.float32)
    wall = np.concatenate([
        np.concatenate([_band(kd, 0), _band(kd, 2), _band(kd, 4)], axis=1),
        np.concatenate([_band(kd, 1), _band(kd, 3), zero], axis=1),
    ], axis=0).astype(np.float16)
    b1 = np.repeat(bias, R).astype(np.float32).reshape(MM, 1)

    nc = _get_module()
    x16 = x.astype(np.float16)
    in_maps = []
    for cr in range(NCORES):
        xs = x16[cr * BPC:(cr + 1) * BPC]
        # pack to [NQ, H, C, 4, W] -> [NQ, H*C, 4*W]
        x4 = np.ascontiguousarray(
            xs.reshape(NQ, 4, C, H, W).transpose(0, 3, 2, 1, 4)
        ).reshape(NQ, H * C, 4 * W)
        in_maps.append({"x": x4, "wall": wall, "b1": b1})
    res = run_bass_kernel_spmd(nc, in_maps, core_ids=list(range(NCORES)))
    global LAST_RESULT
    LAST_RESULT = res

    out = np.empty((B, CO, HO, WO), np.float32)
    for cr in range(NCORES):
        o4 = res.results[cr]["o"].reshape(NQ, CO, HO, 4, WO)
        out[cr * BPC:(cr + 1) * BPC] = (
            o4.transpose(0, 3, 1, 2, 4).reshape(BPC, CO, HO, WO)
        )
    return out


LAST_RESULT = None
